# revision 4
# baseline (speedup 1.0000x reference)
"""BagOfWords embedding-sum kernel for 8 Trainium2 NeuronCores.

Design (HW-probed single-gather-per-token):
  - dma_gather's int16 indices use SIGNED address math with no bounds
    check, and the SWDGE ucode processes every slot up through the LAST
    non-negative index of the list (mid-list negatives gather normally at
    base + idx*stride; trailing negatives are skipped). With the gather
    base at table row 32768, idx = x - 32768 reaches all 50000 rows in
    ONE 768-B descriptor per token (f16 row, 384 gathered elems) - half
    the traffic of a pair-gather. Each 1024-slot call ends in one pad
    slot pointing at an all-zero table row (a valid, positive index) so
    the scan always covers the whole list.
  - Table rows live at 1280-B pitch (elem_step=640) - measurably faster
    HBM random reads than 768-B pitch. Row 1 duplicates row 0 so the
    reference's token-1 -> row-0 remap costs zero device work, and the
    host sends (x - 32768) int16 directly (pure layout + constant shift)
    so the device does no gather-index math at all.
  - 4 SWDGE queues + single_packet=False: packet-granular round-robin
    across queues hides HBM random-read latency (~2x over single-packet,
    ~2.4x over 2 queues).
  - Gathers run in pairs into one double tile, 9 tiles deep: 5 contiguous
    f16/f32 DVE fold ops per 2048 tokens. DVE tensor_tensor grabs the
    shared SBUF port pair and stalls SWDGE descriptor generation, so
    fewer, larger fold ops + deep gather buffering hide the fold.
  - The gpsimd instruction stream stays pure gather-generation: all
    dependent ops (row-127 fixup accumulate-DMAs, scale, stores) are
    emitted at the very end, because the in-order engine would stall
    later descriptor generation behind them.
  - Per 128-row block, 16 main calls cover rows 0..126 fully and row 127
    loses its col-7 slot to the terminator pad; one leftover call
    (call 0) gathers those 4x16 tokens on partitions {2b, 2b+1}, and
    SWDGE accumulate-DMAs (CCE add) fold them into partition 127 of each
    block accumulator (DVE cannot address partition 127 alone).
  - Counts/scale: per-row nonzero counts and the gated reciprocal run on
    device from a separate batch-partition copy of x.
"""

import numpy as np

import concourse.bacc as bacc
import concourse.mybir as mybir
from concourse.tile import TileContext
from concourse.bass_utils import run_bass_kernel_spmd

V, D, B, L = 50000, 300, 4096, 128
DP = 384                 # gathered f16 elems per row (768 B, mult of 256 B)
STEP = 640               # table row pitch in f16 elems (1280 B)
VP = V + 1               # + zero row at V
BASE = 32768             # gather base row; device idx = x - BASE (int16)
PADRAW = V               # raw pad token value -> zero row, idx' = V - BASE > 0
NC = 8
BS = B // NC             # 512 batch rows per core
NBLK = BS // 128         # 4
NI = 1024                # idxs per gather call (hard SWDGE ring limit)
ICOLS = NI // 16         # 64 wrapped idx cols per call
CPB = 16                 # main calls per block
NCALL = NBLK * CPB + 1   # call 0 = leftover, 1..64 = main
NQ = 4                   # SWDGE queues
GBUFS = 9                # gather double-tile pipeline depth

_CACHE = {}


def _build(reps=1):
    key = ("nc", reps)
    if key in _CACHE:
        return _CACHE[key]
    nc = bacc.Bacc("TRN2", target_bir_lowering=False, num_swdge_queues=NQ)
    x_lo = nc.dram_tensor("x_lo", [BS, L], mybir.dt.int32, kind="ExternalInput")
    xq = nc.dram_tensor("xq", [128, NCALL * ICOLS], mybir.dt.int16,
                        kind="ExternalInput")
    embp = nc.dram_tensor("embp", [VP, STEP], mybir.dt.float16,
                          kind="ExternalInput")
    y = nc.dram_tensor("y", [BS, D], mybir.dt.float32, kind="ExternalOutput")

    i32, i16, f16, f32 = (mybir.dt.int32, mybir.dt.int16,
                          mybir.dt.float16, mybir.dt.float32)
    Alu = mybir.AluOpType

    with TileContext(nc) as tc:
        with (
            tc.tile_pool(name="idx", bufs=1) as ip,
            tc.tile_pool(name="small", bufs=1) as sp,
            tc.tile_pool(name="acc", bufs=1) as ap_,
            tc.tile_pool(name="g", bufs=GBUFS) as gp,
            tc.tile_pool(name="m", bufs=2) as mp,
        ):
            # ---- load x in batch-partition layout (counts only) ----
            xt = sp.tile([128, NBLK * L], i32)
            nc.sync.dma_start(
                xt[:].rearrange("p (blk t) -> p blk t", t=L),
                x_lo[:].rearrange("(blk p) t -> p blk t", p=128),
            )
            # ---- wrapped gather indices: host sends (x - BASE) int16
            # directly (pure layout + constant shift; token-1 remap is the
            # table's row-1 duplicate), so no device index math at all.
            NCHUNK = 4
            CCOLS = NCALL * ICOLS // NCHUNK
            idx_w = ip.tile([128, NCALL * ICOLS], i16)
            for ch in range(NCHUNK):
                c0 = ch * CCOLS
                nc.sync.dma_start(idx_w[:, c0:c0 + CCOLS],
                                  xq[:, c0:c0 + CCOLS])

            # ---- counts and gated reciprocal [p, blk] ----
            ne1 = sp.tile([128, NBLK * L], i32)
            nc.vector.tensor_scalar(ne1[:], xt[:], 1, None, Alu.not_equal)
            x2 = sp.tile([128, NBLK * L], i32)
            nc.vector.tensor_tensor(x2[:], xt[:], ne1[:], Alu.mult)
            eq0 = sp.tile([128, NBLK * L], f32)
            nc.vector.tensor_scalar(eq0[:], x2[:], 0, None, Alu.is_equal)
            zc = sp.tile([128, NBLK], f32)
            nc.vector.tensor_reduce(
                zc[:], eq0[:].rearrange("p (blk t) -> p blk t", t=L),
                mybir.AxisListType.X, Alu.add,
            )
            cnt = sp.tile([128, NBLK], f32)
            nc.vector.tensor_scalar(cnt[:], zc[:], -1.0, float(L),
                                    Alu.mult, Alu.add)
            cmax = sp.tile([128, NBLK], f32)
            nc.vector.tensor_scalar(cmax[:], cnt[:], 1.0, None, Alu.max)
            rec = sp.tile([128, NBLK], f32)
            nc.vector.reciprocal(rec[:], cmax[:])
            gate = sp.tile([128, NBLK], f32)
            nc.vector.tensor_scalar(gate[:], cnt[:], 1.0, None, Alu.min)
            rg = sp.tile([128, NBLK], f32)
            nc.vector.tensor_tensor(rg[:], rec[:], gate[:], Alu.mult)

            # ---- gather + blind fold ----
            accs = [ap_.tile([128, DP], f32, name=f"acc{b}", tag=f"acc{b}")
                    for b in range(NBLK)]
            lv = ap_.tile([128, DP], f32, name="lv", tag="lv")
            for _rep in range(reps):
                # leftover call 0 first so lv is long done before the tail
                lg = ap_.tile([128, 8 * DP], f16, name="lvg", tag="lvg")
                sl = idx_w[:, 0:ICOLS]
                nc.gpsimd.dma_gather(
                    lg[:].rearrange("p (c e) -> p c e", e=DP),
                    embp[BASE:, :DP], sl, NI, NI, DP, elem_step=STEP,
                    queue_num=0, single_packet=False,
                )
                h = 4 * DP
                nc.vector.tensor_tensor(
                    lg[:, :h], lg[:, :h], lg[:, h:2 * h], Alu.add)
                nc.vector.tensor_tensor(
                    lg[:, :2 * DP], lg[:, :2 * DP], lg[:, 2 * DP:4 * DP],
                    Alu.add)
                nc.vector.tensor_tensor(
                    lv[:], lg[:, :DP], lg[:, DP:2 * DP], Alu.add)
                for pair in range(NBLK * CPB // 2):
                    blk, kk = divmod(pair, CPB // 2)
                    g = gp.tile([128, 16 * DP], f16, tag="g2")
                    for h_ in range(2):
                        call = 1 + 2 * pair + h_
                        sl = idx_w[:, call * ICOLS:(call + 1) * ICOLS]
                        nc.gpsimd.dma_gather(
                            g[:, h_ * 8 * DP:(h_ + 1) * 8 * DP]
                            .rearrange("p (c e) -> p c e", e=DP),
                            embp[BASE:, :DP], sl, NI, NI, DP,
                            elem_step=STEP,
                            queue_num=call % NQ, single_packet=False,
                        )
                    half = 8 * DP
                    while half >= 2 * DP:
                        nc.vector.tensor_tensor(
                            g[:, :half], g[:, :half],
                            g[:, half:2 * half], Alu.add)
                        half //= 2
                    fold = mp.tile([128, DP], f32, tag="m")
                    nc.vector.tensor_tensor(
                        fold[:], g[:, :DP], g[:, DP:2 * DP], Alu.add)
                    if kk == 0:
                        nc.vector.tensor_copy(accs[blk][:], fold[:])
                    else:
                        nc.vector.tensor_tensor(
                            accs[blk][:], accs[blk][:], fold[:], Alu.add)
                # ---- tail: row-127 fixups (CCE accumulate-DMA), scale,
                # store. Kept off the gather stream until the very end: the
                # in-order gpsimd engine would stall later descriptor
                # generation behind these dependent ops.
                for b in range(NBLK):
                    nc.gpsimd.dma_start(accs[b][127:128, :],
                                        lv[2 * b:2 * b + 1, :],
                                        accum_op=Alu.add)
                    nc.gpsimd.dma_start(accs[b][127:128, :],
                                        lv[2 * b + 1:2 * b + 2, :],
                                        accum_op=Alu.add)
                for b in range(NBLK):
                    nc.vector.tensor_scalar(accs[b][:], accs[b][:],
                                            rg[:, b:b + 1], None, Alu.mult)
                    nc.sync.dma_start(y[b * 128:(b + 1) * 128, :],
                                      accs[b][:, :D])
    nc.compile()
    _CACHE[key] = nc
    return nc


def _token_layout():
    """Slot -> (shard_row, token) map per call; -1 marks pads.

    Pure layout (data-independent). Call 0 is the leftover call; calls
    1..64 are the main calls (16 per 128-row block).
    """
    rows = np.full((NCALL, NI), -1, dtype=np.int64)
    toks = np.full((NCALL, NI), -1, dtype=np.int64)
    s = np.arange(NI)
    p, c = s % 128, s // 128
    for b in range(NBLK):
        for kk in range(CPB):
            call = 1 + b * CPB + kk
            main = p < 127
            rows[call, main] = b * 128 + p[main]
            toks[call, main] = 8 * kk + c[main]
            r127 = (p == 127) & (c < 7)
            rows[call, r127] = b * 128 + 127
            toks[call, r127] = 7 * kk + c[r127]
    # leftover call 0: block b tokens 112..127 of row 127 at parts {2b,2b+1}
    lo = p < 2 * NBLK
    bb = p[lo] // 2
    j = (p[lo] % 2) * 8 + c[lo]
    rows[0, lo] = bb * 128 + 127
    toks[0, lo] = 112 + j
    return rows, toks


_LAYOUT = _token_layout()


def _marshal(x, emb):
    """Host-side layout marshalling (no data-dependent compute)."""
    x = np.asarray(x)
    if x.dtype == np.int64:
        x_lo_full = np.ascontiguousarray(
            x.view(np.int32).reshape(B, L, 2)[:, :, 0])
    else:
        x_lo_full = np.ascontiguousarray(x.astype(np.int32))
    embp = np.zeros((VP, STEP), dtype=np.float16)
    embp[:V, :D] = np.asarray(emb, dtype=np.float32).astype(np.float16)
    embp[1, :] = embp[0, :]          # token 1 -> emb[0] without device remap

    rows, toks = _LAYOUT
    pad_mask = rows < 0
    rows_s = np.where(pad_mask, 0, rows)
    toks_s = np.where(pad_mask, 0, toks)

    in_maps = []
    for core in range(NC):
        shard = x_lo_full[core * BS:(core + 1) * BS]      # [512, 128]
        vals = shard[rows_s, toks_s]
        vals = np.where(pad_mask, PADRAW, vals)           # [NCALL, NI]
        # wrap: slot s -> partition s%16, col s//16; tile to 128 partitions
        w = (vals - BASE).reshape(NCALL, ICOLS, 16)
        w = np.transpose(w, (2, 0, 1)).reshape(16, NCALL * ICOLS)
        xqv = np.ascontiguousarray(np.tile(w, (8, 1)).astype(np.int16))
        in_maps.append({"x_lo": np.ascontiguousarray(shard),
                        "xq": xqv, "embp": embp})
    return in_maps


def kernel(x, emb):
    nc = _build()
    in_maps = _marshal(x, emb)
    res = run_bass_kernel_spmd(nc, in_maps, core_ids=list(range(NC)))
    out = np.concatenate([res.results[c]["y"] for c in range(NC)], axis=0)
    return out
